# revision 7
# baseline (speedup 1.0000x reference)
"""Block-diagonal complex-style locally-connected matmul on 8 NeuronCores.

Math (see reference):
  xp   = x[:, :, perm, :]                  # butterfly permute along N=16384
  xr   = xp[:,0].reshape(B, P, 64)         # P = 4096 blocks, 4*R = 64
  xi   = xp[:,1].reshape(B, P, 64)
  y_re = xr @ W_rr + xi @ W_ri             # per-block [B,64]@[64,64]
  y_im = xr @ W_ir + xi @ W_ii

Device formulation: per block p fold the four 64x64 weights into one
  W_big[p] = [[W_rr, W_ir], [W_ri, W_ii]]  # [128, 128], k = [xr|xi] concat
and xcat[b] = [xr|xi]  # [B, 128]; then
  out[o, b] = sum_k W_big[k, o] * xcat[b, k]   (one matmul per block,
  stationary = W_big [K=128, M=128], moving = xcatT [K=128, N=8])

Sharding: block axis P=4096 split across 8 cores (512 blocks each).
Weights are the dominant traffic (32 MB/core, streamed once); x/out are
2 MB each. All device DMAs are >=256 KB with >=2 KB contiguous runs.
"""

import numpy as np

import concourse.bass as bass
import concourse.bacc as bacc
import concourse.tile as tile
from concourse import mybir
from concourse.bass_utils import run_bass_kernel_spmd

B = 8
N = 16384
R = 16
P = 4096            # blocks total
NCORES = 8
PC = P // NCORES    # 512 blocks per core
K = 128             # contraction (4*R re + 4*R im)
O = 128             # output features per block (64 re + 64 im)

CHUNK = 64          # blocks per W-chunk DMA (4 MB)
GROUP = 64          # blocks per PSUM bank (64 * 8 cols = 512 fp32 = 1 bank)

F32 = mybir.dt.float32

_NC_CACHE = None


def _build_bass():
    nc = bacc.Bacc(
        "TRN2", target_bir_lowering=False, debug=False, num_devices=NCORES
    )
    w_dram = nc.declare_dram_parameter("wk", [K, PC * O], F32, isOutput=False)
    x_dram = nc.declare_dram_parameter("xk", [K, PC * B], F32, isOutput=False)
    o_dram = nc.declare_dram_parameter("out", [O, PC * B], F32, isOutput=True)

    n_chunks = PC // CHUNK
    PS_BUFS = 3

    with tile.TileContext(nc) as tc:
        with (
            tc.tile_pool(name="wpool", bufs=3) as wpool,
            tc.tile_pool(name="xpool", bufs=1) as xpool,
            tc.tile_pool(name="opool", bufs=n_chunks) as opool,
            tc.tile_pool(name="ps", bufs=PS_BUFS, space="PSUM") as pspool,
        ):
            x_sb = xpool.tile([K, PC * B], F32)
            nc.sync.dma_start(x_sb[:], x_dram[:])

            outs = []
            for ci in range(n_chunks):
                w_sb = wpool.tile([K, CHUNK * O], F32)
                nc.sync.dma_start(
                    w_sb[:], w_dram[:, ci * CHUNK * O : (ci + 1) * CHUNK * O]
                )
                ps = pspool.tile([O, GROUP * B], F32)  # one PSUM bank
                for j in range(CHUNK):
                    p = ci * CHUNK + j
                    nc.tensor.matmul(
                        ps[:, j * B : (j + 1) * B],
                        w_sb[:, j * O : (j + 1) * O],
                        x_sb[:, p * B : (p + 1) * B],
                    )
                out_sb = opool.tile([O, GROUP * B], F32)
                nc.vector.tensor_copy(out_sb[:], ps[:])
                outs.append(out_sb)
                nc.sync.dma_start(
                    o_dram[:, ci * GROUP * B : (ci + 1) * GROUP * B], out_sb[:]
                )
    nc.compile()
    return nc


def _get_nc():
    global _NC_CACHE
    if _NC_CACHE is None:
        _NC_CACHE = _build_bass()
    return _NC_CACHE


def kernel(x, W_rr, W_ri, W_ir, W_ii, perm_idx):
    x = np.asarray(x, dtype=np.float32)
    perm = np.asarray(perm_idx, dtype=np.int64)

    # ---- host-side shard packing -------------------------------------
    xp = x[:, :, perm, :]                          # [B, 2, N, R]
    xr = xp[:, 0].reshape(B, P, 4 * R)
    xi = xp[:, 1].reshape(B, P, 4 * R)
    xcat = np.concatenate([xr, xi], axis=2)        # [B, P, 128]
    XT = np.ascontiguousarray(xcat.transpose(2, 1, 0))  # [128 k, P, B]

    wtop = np.concatenate([W_rr, W_ir], axis=2)    # [P, 64, 128]
    wbot = np.concatenate([W_ri, W_ii], axis=2)    # [P, 64, 128]
    wbig = np.concatenate([wtop, wbot], axis=1)    # [P, 128 k, 128 o]
    WK = np.ascontiguousarray(wbig.transpose(1, 0, 2))  # [128 k, P, 128 o]

    in_maps = []
    for c in range(NCORES):
        sl = slice(c * PC, (c + 1) * PC)
        in_maps.append(
            {
                "wk": np.ascontiguousarray(WK[:, sl, :]).reshape(K, PC * O),
                "xk": np.ascontiguousarray(XT[:, sl, :]).reshape(K, PC * B),
            }
        )

    nc = _get_nc()
    res = run_bass_kernel_spmd(nc, in_maps, list(range(NCORES)))

    # ---- gather / unshard --------------------------------------------
    outs = np.stack([np.asarray(res.results[c]["out"]) for c in range(NCORES)])
    outs = outs.reshape(NCORES, O, PC, B)          # [c, o, p_local, b]
    ycat = outs.transpose(3, 0, 2, 1).reshape(B, P, O)  # [b, p, o]
    y_re = ycat[:, :, : 4 * R].reshape(B, N, R)
    y_im = ycat[:, :, 4 * R :].reshape(B, N, R)
    y = np.stack([y_re, y_im], axis=1)             # [B, 2, N, R]
    return np.ascontiguousarray(y, dtype=np.float32)


# revision 8
# speedup vs baseline: 2.2240x; 2.2240x over previous
"""Block-diagonal complex-style locally-connected matmul on 8 NeuronCores.

Math (see reference):
  xp   = x[:, :, perm, :]                  # butterfly permute along N=16384
  xr   = xp[:,0].reshape(B, P, 64)         # P = 4096 blocks, 4*R = 64
  xi   = xp[:,1].reshape(B, P, 64)
  y_re = xr @ W_rr + xi @ W_ri             # per-block [B,64]@[64,64]
  y_im = xr @ W_ir + xi @ W_ii

Device formulation: per block p fold the four 64x64 weights into one
  W_big[p] = [[W_rr, W_ir], [W_ri, W_ii]]  # [128, 128], k = [xr|xi] concat
and xcat[b] = [xr|xi]  # [B, 128]; then per block
  y[b, o] = sum_k xcat[b, k] * W_big[k, o]

PE mapping: weights are 16x the volume of x, so W streams as the MOVING
operand and x is stationary.  Two blocks are packed per matmul:
  lhsT = [x_2i | x_2i+1]  [K=128, M=16],  rhs = [W_2i | W_2i+1]  [K=128, N=256]
  out[0:8, 0:128] = y_2i,  out[8:16, 128:256] = y_2i+1  (other half garbage)
Eight pairs share one PSUM bank, at partition offsets 32g (g=0..3, via
tile_position) x column halves, so the PSUM->SBUF copy and the out DMA
run at full 128-partition width.  Host strips the garbage.

Sharding: block axis P=4096 split across 8 cores (512 blocks each).
Weights are the dominant traffic (32 MB/core, streamed once).
"""

import numpy as np

import concourse.bass as bass
import concourse.bacc as bacc
import concourse.tile as tile
from concourse import mybir
from concourse.bass_utils import run_bass_kernel_spmd

B = 8
N = 16384
R = 16
P = 4096            # blocks total
NCORES = 8
PC = P // NCORES    # 512 blocks per core
K = 128             # contraction (4*R re + 4*R im)
O = 128             # output features per block (64 re + 64 im)

CHUNK = 64          # blocks per W-chunk DMA (4 MB)
PAIRS_PER_BANK = 8  # 4 partition groups x 2 column halves
NBANKS = PC // 2 // PAIRS_PER_BANK          # 32 bank-rounds per core
BANKS_PER_STAGE = 8                          # staging = 8 banks = 2 MB out DMA

F32 = mybir.dt.float32

_NC_CACHE = None


def _build_bass():
    nc = bacc.Bacc(
        "TRN2", target_bir_lowering=False, debug=False, num_devices=NCORES
    )
    w_dram = nc.declare_dram_parameter("wk", [K, PC * O], F32, isOutput=False)
    x_dram = nc.declare_dram_parameter("xk", [K, PC * B], F32, isOutput=False)
    o_dram = nc.declare_dram_parameter("out", [K, NBANKS * 512], F32, isOutput=True)

    n_chunks = PC // CHUNK
    banks_per_chunk = CHUNK // 2 // PAIRS_PER_BANK   # 4

    with tile.TileContext(nc) as tc:
        with (
            tc.tile_pool(name="wpool", bufs=3) as wpool,
            tc.tile_pool(name="xpool", bufs=1) as xpool,
            tc.tile_pool(name="stg", bufs=2) as stgpool,
            tc.tile_pool(name="ps", bufs=3, space="PSUM") as pspool,
        ):
            x_sb = xpool.tile([K, PC * B], F32)
            nc.sync.dma_start(x_sb[:], x_dram[:])

            stage = None
            for ci in range(n_chunks):
                w_sb = wpool.tile([K, CHUNK * O], F32)
                nc.sync.dma_start(
                    w_sb[:], w_dram[:, ci * CHUNK * O : (ci + 1) * CHUNK * O]
                )
                for b in range(banks_per_chunk):
                    r = ci * banks_per_chunk + b        # global bank index
                    if r % BANKS_PER_STAGE == 0:
                        stage = stgpool.tile([K, BANKS_PER_STAGE * 512], F32)
                    ps = pspool.tile([K, 512], F32)     # one PSUM bank
                    for s in range(PAIRS_PER_BANK):
                        pair = r * PAIRS_PER_BANK + s
                        lp = pair - ci * (CHUNK // 2)   # pair local to chunk
                        g = s % 4
                        h = s // 4
                        nc.tensor.matmul(
                            ps[32 * g : 32 * g + 16, 256 * h : 256 * h + 256],
                            x_sb[:, pair * 16 : (pair + 1) * 16],
                            w_sb[:, lp * 256 : (lp + 1) * 256],
                            tile_position=(0, 32 * g),
                        )
                    ro = r % BANKS_PER_STAGE
                    nc.vector.tensor_copy(
                        stage[:, ro * 512 : (ro + 1) * 512], ps[:]
                    )
                    if ro == BANKS_PER_STAGE - 1:
                        r0 = r - (BANKS_PER_STAGE - 1)
                        nc.sync.dma_start(
                            o_dram[:, r0 * 512 : (r + 1) * 512], stage[:]
                        )
    nc.compile()
    return nc


def _get_nc():
    global _NC_CACHE
    if _NC_CACHE is None:
        _NC_CACHE = _build_bass()
    return _NC_CACHE


def _pack_inputs(x, W_rr, W_ri, W_ir, W_ii, perm_idx):
    x = np.asarray(x, dtype=np.float32)
    perm = np.asarray(perm_idx, dtype=np.int64)

    xp = x[:, :, perm, :]                          # [B, 2, N, R]
    xr = xp[:, 0].reshape(B, P, 4 * R)
    xi = xp[:, 1].reshape(B, P, 4 * R)
    xcat = np.concatenate([xr, xi], axis=2)        # [B, P, 128]
    XT = np.ascontiguousarray(xcat.transpose(2, 1, 0))  # [128 k, P, B]

    wtop = np.concatenate([W_rr, W_ir], axis=2)    # [P, 64, 128]
    wbot = np.concatenate([W_ri, W_ii], axis=2)    # [P, 64, 128]
    wbig = np.concatenate([wtop, wbot], axis=1)    # [P, 128 k, 128 o]
    WK = np.ascontiguousarray(wbig.transpose(1, 0, 2))  # [128 k, P, 128 o]

    in_maps = []
    for c in range(NCORES):
        sl = slice(c * PC, (c + 1) * PC)
        in_maps.append(
            {
                "wk": np.ascontiguousarray(WK[:, sl, :]).reshape(K, PC * O),
                "xk": np.ascontiguousarray(XT[:, sl, :]).reshape(K, PC * B),
            }
        )
    return in_maps


def _unpack_outputs(res):
    ycat = np.empty((B, P, O), dtype=np.float32)   # [b, p, o]
    for c in range(NCORES):
        Oc = np.asarray(res.results[c]["out"]).reshape(K, NBANKS, 2, 256)
        for g in range(4):
            for h in range(2):
                pr = np.arange(NBANKS) * PAIRS_PER_BANK + g + 4 * h
                blk = c * PC + 2 * pr
                ycat[:, blk, :] = Oc[
                    32 * g : 32 * g + 8, :, h, 0:128
                ].transpose(0, 1, 2)
                ycat[:, blk + 1, :] = Oc[
                    32 * g + 8 : 32 * g + 16, :, h, 128:256
                ]
    y_re = ycat[:, :, : 4 * R].reshape(B, N, R)
    y_im = ycat[:, :, 4 * R :].reshape(B, N, R)
    y = np.stack([y_re, y_im], axis=1)             # [B, 2, N, R]
    return np.ascontiguousarray(y, dtype=np.float32)


def kernel(x, W_rr, W_ri, W_ir, W_ii, perm_idx):
    in_maps = _pack_inputs(x, W_rr, W_ri, W_ir, W_ii, perm_idx)
    nc = _get_nc()
    res = run_bass_kernel_spmd(nc, in_maps, list(range(NCORES)))
    return _unpack_outputs(res)


# revision 13
# speedup vs baseline: 2.2329x; 1.0040x over previous
"""Block-diagonal complex-style locally-connected matmul on 8 NeuronCores.

Math (see reference):
  xp   = x[:, :, perm, :]                  # butterfly permute along N=16384
  xr   = xp[:,0].reshape(B, P, 64)         # P = 4096 blocks, 4*R = 64
  xi   = xp[:,1].reshape(B, P, 64)
  y_re = xr @ W_rr + xi @ W_ri             # per-block [B,64]@[64,64]
  y_im = xr @ W_ir + xi @ W_ii

Device formulation: per block p fold the four 64x64 weights into one
  W_big[p] = [[W_rr, W_ir], [W_ri, W_ii]]  # [128, 128], k = [xr|xi] concat
and xcat[b] = [xr|xi]  # [B, 128]; then per block
  y[b, o] = sum_k xcat[b, k] * W_big[k, o]

PE mapping: weights are 16x the volume of x, so W streams as the MOVING
operand and x is stationary.  Two blocks are packed per matmul:
  lhsT = [x_2i | x_2i+1]  [K=128, M=16],  rhs = [W_2i | W_2i+1]  [K=128, N=256]
  out[0:8, 0:128] = y_2i,  out[8:16, 128:256] = y_2i+1  (other half garbage)
Eight pairs share one PSUM bank, at partition offsets 32g (g=0..3, via
tile_position) x column halves, so the PSUM->SBUF copy and the out DMA
run at full 128-partition width.  Host strips the garbage.

Sharding: block axis P=4096 split across 8 cores (512 blocks each).
Weights are the dominant traffic (32 MB/core, streamed once).
"""

import numpy as np

import concourse.bass as bass
import concourse.bacc as bacc
import concourse.tile as tile
from concourse import mybir
from concourse.bass_utils import run_bass_kernel_spmd

B = 8
N = 16384
R = 16
P = 4096            # blocks total
NCORES = 8
PC = P // NCORES    # 512 blocks per core
K = 128             # contraction (4*R re + 4*R im)
O = 128             # output features per block (64 re + 64 im)

CHUNK = 64          # blocks per W-chunk DMA (4 MB)
PAIRS_PER_BANK = 8  # 4 partition groups x 2 column halves
NBANKS = PC // 2 // PAIRS_PER_BANK          # 32 bank-rounds per core
BANKS_PER_STAGE = 16                         # staging = 16 banks (32 KB/part)

F32 = mybir.dt.float32

_NC_CACHE = None


def _build_bass():
    nc = bacc.Bacc(
        "TRN2", target_bir_lowering=False, debug=False, num_devices=NCORES
    )
    w_dram = nc.declare_dram_parameter("wk", [K, PC * O], F32, isOutput=False)
    x_dram = nc.declare_dram_parameter("xk", [K, PC * B], F32, isOutput=False)
    # out[g, eo, b, r, h, o]: pair p = r*8 + g + 4h; eo=0 -> block 2p
    # (even rows of the slot), eo=1 -> block 2p+1.  Garbage never leaves SBUF.
    o_dram = nc.declare_dram_parameter(
        "out", [4, 2, B, NBANKS, 2, O], F32, isOutput=True
    )

    n_chunks = PC // CHUNK
    banks_per_chunk = CHUNK // 2 // PAIRS_PER_BANK   # 4

    with tile.TileContext(nc) as tc:
        with (
            tc.tile_pool(name="wpool", bufs=2) as wpool,
            tc.tile_pool(name="xpool", bufs=1) as xpool,
            tc.tile_pool(name="stg", bufs=2) as stgpool,
            tc.tile_pool(name="ps", bufs=3, space="PSUM") as pspool,
        ):
            x_sb = xpool.tile([K, PC * B], F32)
            nc.sync.dma_start(x_sb[:], x_dram[:])

            stage = None
            for ci in range(n_chunks):
                w_sb = wpool.tile([K, CHUNK * O], F32)
                nc.sync.dma_start(
                    w_sb[:], w_dram[:, ci * CHUNK * O : (ci + 1) * CHUNK * O]
                )
                for b in range(banks_per_chunk):
                    r = ci * banks_per_chunk + b        # global bank index
                    if r % BANKS_PER_STAGE == 0:
                        stage = stgpool.tile([K, BANKS_PER_STAGE * 512], F32)
                    ps = pspool.tile([K, 512], F32)     # one PSUM bank
                    for s in range(PAIRS_PER_BANK):
                        pair = r * PAIRS_PER_BANK + s
                        lp = pair - ci * (CHUNK // 2)   # pair local to chunk
                        g = s % 4
                        h = s // 4
                        nc.tensor.matmul(
                            ps[32 * g : 32 * g + 16, 256 * h : 256 * h + 256],
                            x_sb[:, pair * 16 : (pair + 1) * 16],
                            w_sb[:, lp * 256 : (lp + 1) * 256],
                            tile_position=(0, 32 * g),
                        )
                    ro = r % BANKS_PER_STAGE
                    nc.vector.tensor_copy(
                        stage[:, ro * 512 : (ro + 1) * 512], ps[:]
                    )
                    if ro == BANKS_PER_STAGE - 1:
                        r0 = r - (BANKS_PER_STAGE - 1)
                        # ship only the useful quadrants: for each partition
                        # group g, rows 32g..+8 hold even-block rows (cols
                        # 0:128 of every 256) and rows 32g+8..+16 hold
                        # odd-block rows (cols 128:256).
                        for g in range(4):
                            for eo in range(2):
                                src = stage[
                                    32 * g + 8 * eo : 32 * g + 8 * eo + 8, :
                                ].rearrange("p (r h c) -> p r h c", h=2, c=256)[
                                    :, :, :, eo * 128 : (eo + 1) * 128
                                ]
                                nc.sync.dma_start(
                                    o_dram[
                                        g, eo, :, r0 : r0 + BANKS_PER_STAGE, :, :
                                    ],
                                    src,
                                )
    nc.compile()
    return nc


def _get_nc():
    global _NC_CACHE
    if _NC_CACHE is None:
        _NC_CACHE = _build_bass()
    return _NC_CACHE


def _pack_inputs(x, W_rr, W_ri, W_ir, W_ii, perm_idx):
    x = np.asarray(x, dtype=np.float32)
    perm = np.asarray(perm_idx, dtype=np.int64)

    xp = x[:, :, perm, :]                          # [B, 2, N, R]
    xr = xp[:, 0].reshape(B, P, 4 * R)
    xi = xp[:, 1].reshape(B, P, 4 * R)
    xcat = np.concatenate([xr, xi], axis=2)        # [B, P, 128]
    XT = np.ascontiguousarray(xcat.transpose(2, 1, 0))  # [128 k, P, B]

    wtop = np.concatenate([W_rr, W_ir], axis=2)    # [P, 64, 128]
    wbot = np.concatenate([W_ri, W_ii], axis=2)    # [P, 64, 128]
    wbig = np.concatenate([wtop, wbot], axis=1)    # [P, 128 k, 128 o]
    WK = np.ascontiguousarray(wbig.transpose(1, 0, 2))  # [128 k, P, 128 o]

    in_maps = []
    for c in range(NCORES):
        sl = slice(c * PC, (c + 1) * PC)
        in_maps.append(
            {
                "wk": np.ascontiguousarray(WK[:, sl, :]).reshape(K, PC * O),
                "xk": np.ascontiguousarray(XT[:, sl, :]).reshape(K, PC * B),
            }
        )
    return in_maps


def _unpack_outputs(res):
    ycat = np.empty((B, P, O), dtype=np.float32)   # [b, p, o]
    for c in range(NCORES):
        Oc = np.asarray(res.results[c]["out"]).reshape(4, 2, B, NBANKS, 2, O)
        for g in range(4):
            for h in range(2):
                pr = np.arange(NBANKS) * PAIRS_PER_BANK + g + 4 * h
                blk = c * PC + 2 * pr
                ycat[:, blk, :] = Oc[g, 0, :, :, h, :]
                ycat[:, blk + 1, :] = Oc[g, 1, :, :, h, :]
    y_re = ycat[:, :, : 4 * R].reshape(B, N, R)
    y_im = ycat[:, :, 4 * R :].reshape(B, N, R)
    y = np.stack([y_re, y_im], axis=1)             # [B, 2, N, R]
    return np.ascontiguousarray(y, dtype=np.float32)


def kernel(x, W_rr, W_ri, W_ir, W_ii, perm_idx):
    in_maps = _pack_inputs(x, W_rr, W_ri, W_ir, W_ii, perm_idx)
    nc = _get_nc()
    res = run_bass_kernel_spmd(nc, in_maps, list(range(NCORES)))
    return _unpack_outputs(res)


# revision 17
# speedup vs baseline: 2.2858x; 1.0237x over previous
"""Block-diagonal complex-style locally-connected matmul on 8 NeuronCores.

Math (see reference):
  xp   = x[:, :, perm, :]                  # butterfly permute along N=16384
  xr   = xp[:,0].reshape(B, P, 64)         # P = 4096 blocks, 4*R = 64
  xi   = xp[:,1].reshape(B, P, 64)
  y_re = xr @ W_rr + xi @ W_ri             # per-block [B,64]@[64,64]
  y_im = xr @ W_ir + xi @ W_ii

Device formulation: per block p fold the four 64x64 weights into one
  W_big[p] = [[W_rr, W_ir], [W_ri, W_ii]]  # [128, 128], k = [xr|xi] concat
and xcat[b] = [xr|xi]  # [B, 128]; then per block
  y[b, o] = sum_k xcat[b, k] * W_big[k, o]

PE mapping: weights are 16x the volume of x, so W streams as the MOVING
operand and x is stationary.  Two blocks are packed per matmul:
  lhsT = [x_2i | x_2i+1]  [K=128, M=16],  rhs = [W_2i | W_2i+1]  [K=128, N=256]
  out[0:8, 0:128] = y_2i,  out[8:16, 128:256] = y_2i+1  (other half garbage)
Eight pairs share one PSUM bank, at partition offsets 32g (g=0..3, via
tile_position) x column halves, so the PSUM->SBUF copy and the out DMA
run at full 128-partition width.  Host strips the garbage.

Sharding: block axis P=4096 split across 8 cores (512 blocks each).
Weights are the dominant traffic (32 MB/core, streamed once).
"""

import numpy as np

import concourse.bass as bass
import concourse.bacc as bacc
import concourse.tile as tile
from concourse import mybir
from concourse.bass_utils import run_bass_kernel_spmd

B = 8
N = 16384
R = 16
P = 4096            # blocks total
NCORES = 8
PC = P // NCORES    # 512 blocks per core
K = 128             # contraction (4*R re + 4*R im)
O = 128             # output features per block (64 re + 64 im)

CHUNK = 64          # blocks per W-chunk DMA (4 MB)
PAIRS_PER_BANK = 8  # 4 partition groups x 2 column halves
NBANKS = PC // 2 // PAIRS_PER_BANK          # 32 bank-rounds per core
BANKS_PER_STAGE = 8                          # staging = 8 banks (16 KB/part)

F32 = mybir.dt.float32

_NC_CACHE = None


def _build_bass():
    nc = bacc.Bacc(
        "TRN2", target_bir_lowering=False, debug=False, num_devices=NCORES
    )
    w_dram = nc.declare_dram_parameter("wk", [K, PC * O], F32, isOutput=False)
    x_dram = nc.declare_dram_parameter("xk", [K, PC * B], F32, isOutput=False)
    # out[g, eo, b, r, h, o]: pair p = r*8 + g + 4h; eo=0 -> block 2p
    # (even rows of the slot), eo=1 -> block 2p+1.  Garbage never leaves SBUF.
    o_dram = nc.declare_dram_parameter(
        "out", [4, 2, B, NBANKS, 2, O], F32, isOutput=True
    )

    n_chunks = PC // CHUNK
    banks_per_chunk = CHUNK // 2 // PAIRS_PER_BANK   # 4

    with tile.TileContext(nc) as tc:
        with (
            tc.tile_pool(name="wpool", bufs=3) as wpool,
            tc.tile_pool(name="xpool", bufs=1) as xpool,
            tc.tile_pool(name="stg", bufs=2) as stgpool,
            tc.tile_pool(name="ps", bufs=3, space="PSUM") as pspool,
        ):
            # x + out ride the ACT HWDGE ring so their semaphore waits can't
            # head-of-line block W-chunk descriptor generation on the SP ring.
            x_sb = xpool.tile([K, PC * B], F32)
            nc.scalar.dma_start(x_sb[:], x_dram[:])

            stage = None
            for ci in range(n_chunks):
                w_sb = wpool.tile([K, CHUNK * O], F32)
                nc.sync.dma_start(
                    w_sb[:], w_dram[:, ci * CHUNK * O : (ci + 1) * CHUNK * O]
                )
                for b in range(banks_per_chunk):
                    r = ci * banks_per_chunk + b        # global bank index
                    if r % BANKS_PER_STAGE == 0:
                        stage = stgpool.tile([K, BANKS_PER_STAGE * 512], F32)
                    ps = pspool.tile([K, 512], F32)     # one PSUM bank
                    for s in range(PAIRS_PER_BANK):
                        pair = r * PAIRS_PER_BANK + s
                        lp = pair - ci * (CHUNK // 2)   # pair local to chunk
                        g = s % 4
                        h = s // 4
                        nc.tensor.matmul(
                            ps[32 * g : 32 * g + 16, 256 * h : 256 * h + 256],
                            x_sb[:, pair * 16 : (pair + 1) * 16],
                            w_sb[:, lp * 256 : (lp + 1) * 256],
                            tile_position=(0, 32 * g),
                        )
                    ro = r % BANKS_PER_STAGE
                    nc.vector.tensor_copy(
                        stage[:, ro * 512 : (ro + 1) * 512], ps[:]
                    )
                    if ro == BANKS_PER_STAGE - 1:
                        r0 = r - (BANKS_PER_STAGE - 1)
                        # ship only the useful quadrants: for each partition
                        # group g, rows 32g..+8 hold even-block rows (cols
                        # 0:128 of every 256) and rows 32g+8..+16 hold
                        # odd-block rows (cols 128:256).
                        for g in range(4):
                            for eo in range(2):
                                src = stage[
                                    32 * g + 8 * eo : 32 * g + 8 * eo + 8, :
                                ].rearrange("p (r h c) -> p r h c", h=2, c=256)[
                                    :, :, :, eo * 128 : (eo + 1) * 128
                                ]
                                nc.scalar.dma_start(
                                    o_dram[
                                        g, eo, :, r0 : r0 + BANKS_PER_STAGE, :, :
                                    ],
                                    src,
                                )
    nc.compile()
    return nc


def _get_nc():
    global _NC_CACHE
    if _NC_CACHE is None:
        _NC_CACHE = _build_bass()
    return _NC_CACHE


def _pack_inputs(x, W_rr, W_ri, W_ir, W_ii, perm_idx):
    x = np.asarray(x, dtype=np.float32)
    perm = np.asarray(perm_idx, dtype=np.int64)

    xp = x[:, :, perm, :]                          # [B, 2, N, R]
    xr = xp[:, 0].reshape(B, P, 4 * R)
    xi = xp[:, 1].reshape(B, P, 4 * R)
    xcat = np.concatenate([xr, xi], axis=2)        # [B, P, 128]
    XT = np.ascontiguousarray(xcat.transpose(2, 1, 0))  # [128 k, P, B]

    wtop = np.concatenate([W_rr, W_ir], axis=2)    # [P, 64, 128]
    wbot = np.concatenate([W_ri, W_ii], axis=2)    # [P, 64, 128]
    wbig = np.concatenate([wtop, wbot], axis=1)    # [P, 128 k, 128 o]
    WK = np.ascontiguousarray(wbig.transpose(1, 0, 2))  # [128 k, P, 128 o]

    in_maps = []
    for c in range(NCORES):
        sl = slice(c * PC, (c + 1) * PC)
        in_maps.append(
            {
                "wk": np.ascontiguousarray(WK[:, sl, :]).reshape(K, PC * O),
                "xk": np.ascontiguousarray(XT[:, sl, :]).reshape(K, PC * B),
            }
        )
    return in_maps


def _unpack_outputs(res):
    ycat = np.empty((B, P, O), dtype=np.float32)   # [b, p, o]
    for c in range(NCORES):
        Oc = np.asarray(res.results[c]["out"]).reshape(4, 2, B, NBANKS, 2, O)
        for g in range(4):
            for h in range(2):
                pr = np.arange(NBANKS) * PAIRS_PER_BANK + g + 4 * h
                blk = c * PC + 2 * pr
                ycat[:, blk, :] = Oc[g, 0, :, :, h, :]
                ycat[:, blk + 1, :] = Oc[g, 1, :, :, h, :]
    y_re = ycat[:, :, : 4 * R].reshape(B, N, R)
    y_im = ycat[:, :, 4 * R :].reshape(B, N, R)
    y = np.stack([y_re, y_im], axis=1)             # [B, 2, N, R]
    return np.ascontiguousarray(y, dtype=np.float32)


def kernel(x, W_rr, W_ri, W_ir, W_ii, perm_idx):
    in_maps = _pack_inputs(x, W_rr, W_ri, W_ir, W_ii, perm_idx)
    nc = _get_nc()
    res = run_bass_kernel_spmd(nc, in_maps, list(range(NCORES)))
    return _unpack_outputs(res)


# revision 19
# speedup vs baseline: 2.2909x; 1.0022x over previous
"""Block-diagonal complex-style locally-connected matmul on 8 NeuronCores.

Math (see reference):
  xp   = x[:, :, perm, :]                  # butterfly permute along N=16384
  xr   = xp[:,0].reshape(B, P, 64)         # P = 4096 blocks, 4*R = 64
  xi   = xp[:,1].reshape(B, P, 64)
  y_re = xr @ W_rr + xi @ W_ri             # per-block [B,64]@[64,64]
  y_im = xr @ W_ir + xi @ W_ii

Device formulation: per block p fold the four 64x64 weights into one
  W_big[p] = [[W_rr, W_ir], [W_ri, W_ii]]  # [128, 128], k = [xr|xi] concat
and xcat[b] = [xr|xi]  # [B, 128]; then per block
  y[b, o] = sum_k xcat[b, k] * W_big[k, o]

PE mapping: weights are 16x the volume of x, so W streams as the MOVING
operand and x is stationary.  Two blocks are packed per matmul:
  lhsT = [x_2i | x_2i+1]  [K=128, M=16],  rhs = [W_2i | W_2i+1]  [K=128, N=256]
  out[0:8, 0:128] = y_2i,  out[8:16, 128:256] = y_2i+1  (other half garbage)
Eight pairs share one PSUM bank, at partition offsets 32g (g=0..3, via
tile_position) x column halves, so the PSUM->SBUF copy and the out DMA
run at full 128-partition width.  Host strips the garbage.

Sharding: block axis P=4096 split across 8 cores (512 blocks each).
Weights are the dominant traffic (32 MB/core, streamed once).
"""

import numpy as np

import concourse.bass as bass
import concourse.bacc as bacc
import concourse.tile as tile
from concourse import mybir
from concourse.bass_utils import run_bass_kernel_spmd

B = 8
N = 16384
R = 16
P = 4096            # blocks total
NCORES = 8
PC = P // NCORES    # 512 blocks per core
K = 128             # contraction (4*R re + 4*R im)
O = 128             # output features per block (64 re + 64 im)

CHUNK = 32          # blocks per W-chunk DMA (2 MB)
PAIRS_PER_BANK = 8  # 4 partition groups x 2 column halves
NBANKS = PC // 2 // PAIRS_PER_BANK          # 32 bank-rounds per core
BANKS_PER_STAGE = 8                          # staging = 8 banks (16 KB/part)

F32 = mybir.dt.float32

_NC_CACHE = None


def _build_bass():
    nc = bacc.Bacc(
        "TRN2", target_bir_lowering=False, debug=False, num_devices=NCORES
    )
    w_dram = nc.declare_dram_parameter("wk", [K, PC * O], F32, isOutput=False)
    x_dram = nc.declare_dram_parameter("xk", [K, PC * B], F32, isOutput=False)
    # out[g, eo, b, r, h, o]: pair p = r*8 + g + 4h; eo=0 -> block 2p
    # (even rows of the slot), eo=1 -> block 2p+1.  Garbage never leaves SBUF.
    o_dram = nc.declare_dram_parameter(
        "out", [4, 2, B, NBANKS, 2, O], F32, isOutput=True
    )

    n_chunks = PC // CHUNK
    banks_per_chunk = CHUNK // 2 // PAIRS_PER_BANK   # 4

    with tile.TileContext(nc) as tc:
        with (
            tc.tile_pool(name="wpool", bufs=6) as wpool,
            tc.tile_pool(name="xpool", bufs=1) as xpool,
            tc.tile_pool(name="stg", bufs=2) as stgpool,
            tc.tile_pool(name="ps", bufs=4, space="PSUM") as pspool,
        ):
            # x + out ride the ACT HWDGE ring so their semaphore waits can't
            # head-of-line block W-chunk descriptor generation on the SP ring.
            x_sb = xpool.tile([K, PC * B], F32)
            nc.scalar.dma_start(x_sb[:], x_dram[:])

            stage = None
            for ci in range(n_chunks):
                w_sb = wpool.tile([K, CHUNK * O], F32)
                nc.sync.dma_start(
                    w_sb[:], w_dram[:, ci * CHUNK * O : (ci + 1) * CHUNK * O]
                )
                for b in range(banks_per_chunk):
                    r = ci * banks_per_chunk + b        # global bank index
                    if r % BANKS_PER_STAGE == 0:
                        stage = stgpool.tile([K, BANKS_PER_STAGE * 512], F32)
                    ps = pspool.tile([K, 512], F32)     # one PSUM bank
                    for s in range(PAIRS_PER_BANK):
                        pair = r * PAIRS_PER_BANK + s
                        lp = pair - ci * (CHUNK // 2)   # pair local to chunk
                        g = s % 4
                        h = s // 4
                        nc.tensor.matmul(
                            ps[32 * g : 32 * g + 16, 256 * h : 256 * h + 256],
                            x_sb[:, pair * 16 : (pair + 1) * 16],
                            w_sb[:, lp * 256 : (lp + 1) * 256],
                            tile_position=(0, 32 * g),
                        )
                    ro = r % BANKS_PER_STAGE
                    nc.vector.tensor_copy(
                        stage[:, ro * 512 : (ro + 1) * 512], ps[:]
                    )
                    if ro == BANKS_PER_STAGE - 1:
                        r0 = r - (BANKS_PER_STAGE - 1)
                        # ship only the useful quadrants: for each partition
                        # group g, rows 32g..+8 hold even-block rows (cols
                        # 0:128 of every 256) and rows 32g+8..+16 hold
                        # odd-block rows (cols 128:256).
                        for g in range(4):
                            for eo in range(2):
                                src = stage[
                                    32 * g + 8 * eo : 32 * g + 8 * eo + 8, :
                                ].rearrange("p (r h c) -> p r h c", h=2, c=256)[
                                    :, :, :, eo * 128 : (eo + 1) * 128
                                ]
                                nc.scalar.dma_start(
                                    o_dram[
                                        g, eo, :, r0 : r0 + BANKS_PER_STAGE, :, :
                                    ],
                                    src,
                                )
    nc.compile()
    return nc


def _get_nc():
    global _NC_CACHE
    if _NC_CACHE is None:
        _NC_CACHE = _build_bass()
    return _NC_CACHE


def _pack_inputs(x, W_rr, W_ri, W_ir, W_ii, perm_idx):
    x = np.asarray(x, dtype=np.float32)
    perm = np.asarray(perm_idx, dtype=np.int64)

    xp = x[:, :, perm, :]                          # [B, 2, N, R]
    xr = xp[:, 0].reshape(B, P, 4 * R)
    xi = xp[:, 1].reshape(B, P, 4 * R)
    xcat = np.concatenate([xr, xi], axis=2)        # [B, P, 128]
    XT = np.ascontiguousarray(xcat.transpose(2, 1, 0))  # [128 k, P, B]

    wtop = np.concatenate([W_rr, W_ir], axis=2)    # [P, 64, 128]
    wbot = np.concatenate([W_ri, W_ii], axis=2)    # [P, 64, 128]
    wbig = np.concatenate([wtop, wbot], axis=1)    # [P, 128 k, 128 o]
    WK = np.ascontiguousarray(wbig.transpose(1, 0, 2))  # [128 k, P, 128 o]

    in_maps = []
    for c in range(NCORES):
        sl = slice(c * PC, (c + 1) * PC)
        in_maps.append(
            {
                "wk": np.ascontiguousarray(WK[:, sl, :]).reshape(K, PC * O),
                "xk": np.ascontiguousarray(XT[:, sl, :]).reshape(K, PC * B),
            }
        )
    return in_maps


def _unpack_outputs(res):
    ycat = np.empty((B, P, O), dtype=np.float32)   # [b, p, o]
    for c in range(NCORES):
        Oc = np.asarray(res.results[c]["out"]).reshape(4, 2, B, NBANKS, 2, O)
        for g in range(4):
            for h in range(2):
                pr = np.arange(NBANKS) * PAIRS_PER_BANK + g + 4 * h
                blk = c * PC + 2 * pr
                ycat[:, blk, :] = Oc[g, 0, :, :, h, :]
                ycat[:, blk + 1, :] = Oc[g, 1, :, :, h, :]
    y_re = ycat[:, :, : 4 * R].reshape(B, N, R)
    y_im = ycat[:, :, 4 * R :].reshape(B, N, R)
    y = np.stack([y_re, y_im], axis=1)             # [B, 2, N, R]
    return np.ascontiguousarray(y, dtype=np.float32)


def kernel(x, W_rr, W_ri, W_ir, W_ii, perm_idx):
    in_maps = _pack_inputs(x, W_rr, W_ri, W_ir, W_ii, perm_idx)
    nc = _get_nc()
    res = run_bass_kernel_spmd(nc, in_maps, list(range(NCORES)))
    return _unpack_outputs(res)
